# revision 36
# baseline (speedup 1.0000x reference)
"""DiagonalElman fused Trainium2 kernel.

Math (per batch b):
    x_proj = x @ W_in.T                      # [T, D]
    xw     = x_proj @ W_x.T                  # [T, D]
    alpha  = sigmoid(alpha_raw)              # [D]
    h_t    = tanh(xw_t + alpha * h_{t-1} + b)
    cell_t = h_t * silu(x_proj_t + b_gate)
    out    = cell @ W_out.T                  # [T, D]
    returns (out, h_T)

Sharding: pure batch-parallel SPMD — core k processes batch k. No collectives.

The sequential recurrence is parallelized with a chunked-warmup scheme:
alpha = sigmoid(alpha_raw) makes |d h_t / d h_{t-k}| <= alpha^k, so a chunk
of the sequence recomputed from a zero initial state after W warmup steps
matches the true trajectory to alpha^W (~2e-4 for alpha~0.12, W=4 — far
below the ~1e-2 bf16 matmul noise; chunk 0 is exact via an
atanh(h0)-alpha*h0 pad so h0 enters exactly).  T is split into independent
chunks of L=16 steps, each preceded by W warmup steps; all chunks advance
in lockstep, one vector MAC (scalar_tensor_tensor) + one scalar tanh per
step over a contiguous [128, G/2, C] tile (phase-major slot layout:
slot = s*C + c), h overwriting xw in place.

Layouts: channels e live on partitions (e = g*128 + p), time on the free
axis.  Host pre-transposes x and the weights so every matmul contracts on
partitions with zero on-device transposes:
    mm1: lhsT = W_in.T tile [d, e]   rhs = x.T [d, t]     -> psum [e, t]
    mm2: lhsT = W_x.T tile [e', e2]  rhs = x_proj [e', t] -> psum [e2, t]
    mm3: lhsT = cell tile [e, t]     rhs = W_out.T [e, n] -> psum [t, n]
Matmul I/O is bf16 (full PE rate); psum accumulation and the recurrence
state are fp32.
"""

import numpy as np
import ml_dtypes

# ---------------------------------------------------------------- constants
B = 8          # batch == number of cores
T = 2048       # sequence length
D = 1024       # channels
P = 128        # partitions
G = D // P     # channel groups (free-axis dim)
TP = 1024      # piece length (recurrence processed per piece)
L = 16         # chunk length
W = 4          # warmup steps
TB = 512       # matmul time-block
NP = T // TP   # pieces
C = TP // L    # chunks per piece (per group)
S = L + W      # lockstep steps per piece
PS = S * C     # xwh slot-columns per piece (phase-major: slot = s*C + c)
BUFC = NP * PS

_nc_cache = {}


def _set_dims(T_=2048, TP_=1024, L_=16, W_=4, TB_=512):
    """Recompute module dims — used by test.py for scaled-down validation.
    kernel() always runs the full problem size."""
    global T, TP, L, W, TB, NP, C, S, PS, BUFC
    T, TP, L, W, TB = T_, TP_, L_, W_, TB_
    NP = T // TP
    C = TP // L
    S = L + W
    PS = S * C
    BUFC = NP * PS
    _nc_cache.clear()


def _split_drain_waits(nc, mybir, max_waits=1):
    """This walrus build rejects instructions carrying more than a couple of
    sync waits ("Too many sync wait commands").  Move excess waits onto
    single-wait same-engine NOPs preceding the instruction — engines execute
    their queue in order, so waiting on a preceding NOP is equivalent."""
    for f in nc.m.functions:
        for bb in f.blocks:
            newlist = []
            for ins in bb.instructions:
                si = getattr(ins, "sync_info", None)
                if si is not None and len(si.on_wait) > max_waits:
                    waits = list(si.on_wait)
                    keep = waits[-max_waits:] if ins.opcode != "Drain" else []
                    spill = waits[:-max_waits] if ins.opcode != "Drain" else waits
                    si.on_wait.clear()
                    si.on_wait.extend(keep)
                    for i, w in enumerate(spill):
                        newlist.append(mybir.InstNoOp(
                            name=f"{ins.name}_waitsplit_{i}",
                            engine=ins.engine,
                            ins=[], outs=[],
                            sync_info=mybir.SyncInfo(on_wait=[w], on_update=[]),
                        ))
                newlist.append(ins)
            bb.instructions[:] = newlist


def build_nc(alpha_uniform=True, reps=1):
    """Build the per-core Bass program (same NEFF on all 8 cores).
    reps>1 wraps the whole pipeline in a hardware loop — used only for
    timing (HW time = (wall(reps=N) - wall(reps=1)) / (N-1))."""
    import concourse.bass as bass
    import concourse.mybir as mybir
    import concourse.tile as tile
    import contextlib

    key = ("v1", alpha_uniform, reps)
    if key in _nc_cache:
        return _nc_cache[key]

    f32 = mybir.dt.float32
    bf16 = mybir.dt.bfloat16
    AF = mybir.ActivationFunctionType
    OP = mybir.AluOpType

    nc = bass.Bass()

    # x arrives as [piece, t-block, p, g, t-within-block] (host-baked
    # layout) so each t-block's x loads with one linear DMA
    x_t = nc.dram_tensor("x_t", [NP, TP // TB, P, G, TB], bf16,
                         kind="ExternalInput")
    # weights arrive pre-arranged in SBUF layout [p, g, e] (host bakes the
    # (g p) e -> p g e rearrange) so the load is one linear DMA
    w_in = nc.dram_tensor("w_in", [P, G, D], bf16, kind="ExternalInput")    # W_in.T
    w_x = nc.dram_tensor("w_x", [P, G, D], bf16, kind="ExternalInput")      # W_x.T
    w_out = nc.dram_tensor("w_out", [P, G, D], bf16, kind="ExternalInput")  # W_out.T
    alpha_in = nc.dram_tensor("alpha_in", [P, 1] if alpha_uniform else [P, G],
                              f32, kind="ExternalInput")
    bias_b = nc.dram_tensor("bias_b", [P, G], f32, kind="ExternalInput")
    bias_g = nc.dram_tensor("bias_g", [P, G], f32, kind="ExternalInput")
    init0 = nc.dram_tensor("init0", [P, G, C], f32, kind="ExternalInput")
    pad0 = nc.dram_tensor("pad0", [P, G, W], f32, kind="ExternalInput")
    out = nc.dram_tensor("out", [T, D], f32, kind="ExternalOutput")
    h_fin = nc.dram_tensor("h_fin", [D], f32, kind="ExternalOutput")

    w_in3, w_x3, w_out3 = w_in, w_x, w_out

    with tile.TileContext(nc) as tc:
        with (
            tc.tile_pool(name="big", bufs=1) as big,
            tc.tile_pool(name="wpool", bufs=1) as wpool,
            tc.tile_pool(name="xin", bufs=3) as xin,
            tc.tile_pool(name="small", bufs=1) as small,
            tc.tile_pool(name="tmps", bufs=4) as tmps,
            tc.tile_pool(name="ostg", bufs=3) as ostg,
            tc.tile_pool(name="psum", bufs=8, space="PSUM") as psum,
        ):
            # persistent SBUF state
            xwh = big.tile([P, G, BUFC], f32)       # xw+b, overwritten by h in place
            gbuf = big.tile([P, G, T], bf16)        # x_proj -> silu gate -> cell
            w_in_sb = wpool.tile([P, G, D], bf16)
            w_x_sb = wpool.tile([P, G, D], bf16)
            w_out_sb = wpool.tile([P, G, D], bf16)
            alpha_sb = small.tile([P, 1] if alpha_uniform else [P, G], f32)
            bb_sb = small.tile([P, G], f32)
            bg_sb = small.tile([P, G], f32)
            init0_sb = small.tile([P, G, C], f32)
            zinit_sb = small.tile([P, G, C], f32)

            nc.sync.dma_start(w_in_sb[:], w_in3[:])
            nc.sync.dma_start(alpha_sb[:], alpha_in[:])
            nc.sync.dma_start(bb_sb[:], bias_b[:])
            nc.sync.dma_start(bg_sb[:], bias_g[:])
            nc.sync.dma_start(init0_sb[:], init0[:])
            # pad columns for piece-0 chunk-0 warmup: slots (s, c=0), s<W
            pad_sb = small.tile([P, G, W], f32)
            nc.sync.dma_start(pad_sb[:], pad0[:])
            nc.vector.tensor_copy(xwh[:, :, 0:W * C:C], pad_sb[:])
            nc.vector.memset(zinit_sb[:], 0.0)
            nc.sync.dma_start(w_x_sb[:], w_x3[:])
            nc.sync.dma_start(w_out_sb[:], w_out3[:])

            # Warm the PE HAM clock gate during the startup DMA wait:
            # ~4.5us of dummy matmuls on a zeroed stub lift the PE to
            # 2.4GHz before the first real matmul issues (cold MMs run at
            # 1.2GHz for the first ~3.4us otherwise).  Results land in a
            # rotating psum slot and are never read.
            wstub = small.tile([P, P], bf16)
            nc.vector.memset(wstub[:], 0.0)
            for _ in range(5):
                wps = psum.tile([P, 512], f32, tag="ps")
                for _ in range(8):
                    nc.tensor.matmul(wps[:, 0:P], wstub[:], wstub[:],
                                     start=True, stop=True)

            def recur_units(p):
                # All C chunks advance in lockstep over slot columns
                # (slot = s*C + c, phase-major, so every access below is
                # contiguous).  Each slot is written exactly once: xw/pad by
                # the evictions/warmup copies, then h in place by the tanh.
                pbase = p * PS
                init_sb = init0_sb if p == 0 else zinit_sb
                for s in range(S):
                    for chain in range(2):
                        gs = slice(chain * (G // 2), (chain + 1) * (G // 2))
                        cur = xwh[:, gs, pbase + s * C: pbase + (s + 1) * C]
                        if s == 0:
                            prev = init_sb[:, gs, :]
                        else:
                            prev = xwh[:, gs, pbase + (s - 1) * C: pbase + s * C]
                        if alpha_uniform:
                            nc.vector.scalar_tensor_tensor(
                                cur, prev, alpha_sb[:, 0:1], cur,
                                OP.mult, OP.add)
                        else:
                            tmp = tmps.tile([P, G // 2, C], f32, tag=f"tmp{chain}")
                            ab = alpha_sb[:, gs, None].to_broadcast((P, G // 2, C))
                            nc.vector.tensor_tensor(tmp[:], prev, ab, OP.mult)
                            nc.vector.tensor_tensor(cur, tmp[:], cur, OP.add)
                        nc.scalar.activation(cur, cur, AF.Tanh)
                    yield

            def gslot(p, dgs=None):
                # gbuf viewed as slots: [P, (g), b, c] with b = t%L, c = chunk
                if dgs is None:
                    v = gbuf[:, :, p * TP:(p + 1) * TP]
                    return v.rearrange("p g (b c) -> p g b c", c=C)
                v = gbuf[:, dgs, p * TP:(p + 1) * TP]
                return v.rearrange("p (b c) -> p b c", c=C)

            def mm12_units(p):
                # mm1 (x_proj -> gbuf slots) and mm2 (xw + b -> xwh slots)
                pbase = p * PS
                AB = TB // L          # chunks per t-block
                for tb in range(TP // TB):
                    xs = xin.tile([P, G, TB], bf16, tag="xs")
                    # gpsimd SWDGE path: keeps x loads off the sync queue,
                    # which is busy with weight DMAs at startup
                    nc.gpsimd.dma_start(xs[:], x_t[p, tb])
                    for eg in range(G):
                        ps = psum.tile([P, TB], f32, tag="ps")
                        for dg in range(G):
                            nc.tensor.matmul(
                                ps[:], w_in_sb[:, dg, eg * P:(eg + 1) * P],
                                xs[:, dg, :],
                                start=(dg == 0), stop=(dg == G - 1))
                        # scramble t-order psum into slot-order gbuf
                        nc.vector.tensor_copy(
                            gslot(p, eg)[:, :, tb * AB:(tb + 1) * AB],
                            ps[:].rearrange("p (a b) -> p b a", b=L))
                        yield
                    for eg in range(G):
                        ps = psum.tile([P, TB], f32, tag="ps")
                        for dg in range(G):
                            nc.tensor.matmul(
                                ps[:], w_x_sb[:, dg, eg * P:(eg + 1) * P],
                                gslot(p, dg)[:, :, tb * AB:(tb + 1) * AB],
                                start=(dg == 0), stop=(dg == G - 1))
                        # psum columns already slot-ordered; store to data slots
                        dst = xwh[:, eg, pbase + W * C: pbase + S * C]
                        dst = dst.rearrange("p (b c) -> p b c", c=C)
                        nc.vector.tensor_scalar(
                            dst[:, :, tb * AB:(tb + 1) * AB],
                            ps[:].rearrange("p (b a) -> p b a", a=AB),
                            bb_sb[:, eg:eg + 1], None, OP.add)
                        yield
                # fill warmup slots (s<W) of this piece from its own xw data,
                # and chunk-0 warmup of the NEXT piece from this piece's tail
                # (both must happen before the respective recurrences
                # overwrite the source slots with h)
                vw = xwh[:, :, pbase: pbase + PS]
                vw = vw.rearrange("p g (s c) -> p g s c", c=C)
                nc.vector.tensor_copy(vw[:, :, 0:W, 1:C],
                                      vw[:, :, L:L + W, 0:C - 1])
                if p + 1 < NP:
                    nx = xwh[:, :, (p + 1) * PS: (p + 2) * PS]
                    nx = nx.rearrange("p g (s c) -> p g s c", c=C)
                    nc.vector.tensor_copy(nx[:, :, 0:W, 0:1],
                                          vw[:, :, L:L + W, C - 1:C])

            def silu_units(p):
                # gate part 1: gbuf <- silu(x_proj + b_gate); per-group so the
                # per-partition bias slice is uniform
                for gg in range(G):
                    nc.scalar.activation(
                        gbuf[:, gg, p * TP:(p + 1) * TP],
                        gbuf[:, gg, p * TP:(p + 1) * TP],
                        AF.Silu, bias=bg_sb[:, gg:gg + 1])
                    yield

            def mm3_units(p):
                # cell = h * gate (contiguous in slot space, per block so PE
                # starts right after the first block), then out = cell@W_out.T
                # with slot-contiguous lhsT (stationary APs must have a single
                # free dim).  Output rows land in DRAM in slot order; the host
                # unscrambles with a cheap gather.  Evictions alternate V/S.
                pbase = p * PS
                GB = min(2 * P, TP)   # gate block (slots)
                for mt in range(TP // P):
                    k0 = mt * P
                    if k0 % GB == 0:
                        nc.vector.tensor_tensor(
                            gbuf[:, :, p * TP + k0: p * TP + k0 + GB],
                            xwh[:, :, pbase + W * C + k0: pbase + W * C + k0 + GB],
                            gbuf[:, :, p * TP + k0: p * TP + k0 + GB], OP.mult)
                    for nh in range(D // 512):
                        ps = psum.tile([P, 512], f32, tag="ps")
                        for eg in range(G):
                            nc.tensor.matmul(
                                ps[:], gbuf[:, eg, p * TP + k0: p * TP + k0 + P],
                                w_out_sb[:, eg, nh * 512:(nh + 1) * 512],
                                start=(eg == 0), stop=(eg == G - 1))
                        st = ostg.tile([P, 512], f32, tag="ost")
                        if (mt + nh) % 2 == 0:
                            nc.scalar.copy(st[:], ps[:])
                        else:
                            nc.vector.tensor_copy(st[:], ps[:])
                        r0 = p * TP + mt * P
                        nc.sync.dma_start(
                            out[r0:r0 + P, nh * 512:(nh + 1) * 512], st[:])
                        yield

            def pump(gen, n):
                for _ in range(n):
                    if next(gen, "done") == "done":
                        return True
                return False

            loop_cm = tc.For_i(0, reps, 1) if reps > 1 else contextlib.nullcontext()
            with loop_cm:
                # Software pipeline across pieces: PE stream is
                #   mm12(0), mm12(1), mm3(0), mm12(2), mm3(1), ..., mm3(NP-1)
                # while piece p's recurrence/silu (V+S) runs during the PE
                # work emitted alongside it — PE never waits on a recurrence.
                for _ in mm12_units(0):
                    pass
                for p in range(NP):
                    rec = recur_units(p)
                    filler = []
                    if p >= 1:
                        filler.append(mm3_units(p - 1))
                    if p + 1 < NP:
                        filler.append(mm12_units(p + 1))
                    sil = silu_units(p)
                    # interleave: pump a few recurrence steps per filler unit
                    total_filler = (TP // P) * (D // 512) * (1 if p >= 1 else 0) \
                        + 2 * G * (TP // TB) * (1 if p + 1 < NP else 0)
                    per = max(1, -(-S // max(total_filler, 1)))
                    done = False
                    for fg in filler:
                        for _ in fg:
                            if not done:
                                done = pump(rec, per)
                    while not done:
                        done = pump(rec, S)
                    # silu can start as soon as mm2(p) has consumed gbuf
                    for _ in sil:
                        pass
                for _ in mm3_units(NP - 1):
                    pass

            # final hidden state: last valid h column
            nc.sync.dma_start(
                h_fin.rearrange("(g p) -> p g", p=P)[:, :, None],
                xwh[:, :, BUFC - 1:BUFC])

    _split_drain_waits(nc, mybir, max_waits=1)
    _nc_cache[key] = nc
    return nc


def _prep_inputs(x, h0, W_in, W_x, alpha_raw, b, b_gate, W_out):
    """Host-side sharding/layout prep. Returns (in_maps, perm, alpha_uniform)."""
    x = np.asarray(x, np.float32)
    h0 = np.asarray(h0, np.float32)
    W_in = np.asarray(W_in, np.float32)
    W_x = np.asarray(W_x, np.float32)
    alpha_raw = np.asarray(alpha_raw, np.float32)
    b = np.asarray(b, np.float32)
    b_gate = np.asarray(b_gate, np.float32)
    W_out = np.asarray(W_out, np.float32)

    alpha = 1.0 / (1.0 + np.exp(-alpha_raw.astype(np.float64)))
    alpha = alpha.astype(np.float32)

    # Channel permutation so alpha is constant per partition (e = g*128 + p):
    # needed for the single-instruction MAC with a per-partition scalar.
    # alpha[g*128 + p] must be independent of g for each p.
    def uniform_ok(a):
        r = a.reshape(G, P)
        return bool(np.all(r == r[0:1]))

    if uniform_ok(alpha):
        perm = np.arange(D)
        alpha_uniform = True
    else:
        perm = np.argsort(alpha, kind="stable")
        # sorted values laid out p-major: position k -> (p = k // G, g = k % G)
        a_s = alpha[perm]
        if np.all(a_s.reshape(P, G) == a_s.reshape(P, G)[:, 0:1]):
            # reorder perm so that channel e' = g*128 + p gets sorted[p*G + g]
            perm = perm.reshape(P, G).T.reshape(D)
            alpha_uniform = True
        else:
            perm = np.arange(D)
            alpha_uniform = False

    alpha_p = alpha[perm]
    W_in_p = W_in[perm, :]
    W_x_p = W_x[perm, :][:, perm]
    W_out_p = W_out[:, perm]
    b_p = b[perm]
    bg_p = b_gate[perm]
    h0_p = h0[:, perm]

    bf = ml_dtypes.bfloat16

    def wlayout(wT):
        # [d, e] -> [p, g, e] with d = g*P + p, contiguous
        return np.ascontiguousarray(
            wT.reshape(G, P, D).swapaxes(0, 1)).astype(bf)

    w_in_in = wlayout(W_in_p.T)
    w_x_in = wlayout(W_x_p.T)
    w_out_in = wlayout(W_out_p.T)

    if alpha_uniform:
        alpha_dev = np.ascontiguousarray(alpha_p.reshape(G, P).T[:, 0:1])
    else:
        alpha_dev = np.ascontiguousarray(alpha_p.reshape(G, P).T)
    bb_dev = np.ascontiguousarray(b_p.reshape(G, P).T)
    bg_dev = np.ascontiguousarray(bg_p.reshape(G, P).T)

    in_maps = []
    for k in range(B):
        xt = np.ascontiguousarray(x[k].T).astype(bf)          # [D, T]
        nTB = TP // TB
        xt = xt.reshape(G, P, NP, nTB, TB).transpose(2, 3, 1, 0, 4)
        x_t = np.ascontiguousarray(xt)                        # [NP,nTB,P,G,TB]
        h0k = h0_p[k]
        init0 = np.zeros((P, G, C), np.float32)
        init0[:, :, 0] = h0k.reshape(G, P).T
        h0c = np.clip(h0k, -0.9999999, 0.9999999)
        pad_val = np.arctanh(h0c) - alpha_p * h0k          # [D]
        pad0 = np.repeat(pad_val.reshape(G, P).T[:, :, None], W, axis=2)
        in_maps.append({
            "x_t": x_t,
            "w_in": w_in_in, "w_x": w_x_in, "w_out": w_out_in,
            "alpha_in": np.ascontiguousarray(alpha_dev, np.float32),
            "bias_b": np.ascontiguousarray(bb_dev, np.float32),
            "bias_g": np.ascontiguousarray(bg_dev, np.float32),
            "init0": init0,
            "pad0": np.ascontiguousarray(pad0, np.float32),
        })
    return in_maps, perm, alpha_uniform


def kernel(x, h0, W_in, W_x, alpha_raw, b, b_gate, W_out):
    from concourse.bass_utils import run_bass_kernel_spmd

    in_maps, perm, alpha_uniform = _prep_inputs(
        x, h0, W_in, W_x, alpha_raw, b, b_gate, W_out)
    nc = build_nc(alpha_uniform=alpha_uniform)
    res = run_bass_kernel_spmd(nc, in_maps, core_ids=list(range(B)))

    # device "out" rows are in slot order: row p*TP + b*C + c holds
    # t = p*TP + c*L + b.  Build the gather index t -> row once.
    tt = np.arange(T)
    pp, rr = tt // TP, tt % TP
    rows = pp * TP + (rr % L) * C + rr // L
    output = np.stack([res.results[k]["out"][rows] for k in range(B)], axis=0)
    hf = np.empty((B, D), np.float32)
    for k in range(B):
        hf[k, perm] = res.results[k]["h_fin"]
    return np.ascontiguousarray(output, dtype=np.float32), hf


# revision 37
# speedup vs baseline: 1.0020x; 1.0020x over previous
"""DiagonalElman fused Trainium2 kernel.

Math (per batch b):
    x_proj = x @ W_in.T                      # [T, D]
    xw     = x_proj @ W_x.T                  # [T, D]
    alpha  = sigmoid(alpha_raw)              # [D]
    h_t    = tanh(xw_t + alpha * h_{t-1} + b)
    cell_t = h_t * silu(x_proj_t + b_gate)
    out    = cell @ W_out.T                  # [T, D]
    returns (out, h_T)

Sharding: pure batch-parallel SPMD — core k processes batch k. No collectives.

The sequential recurrence is parallelized with a chunked-warmup scheme:
alpha = sigmoid(alpha_raw) makes |d h_t / d h_{t-k}| <= alpha^k, so a chunk
of the sequence recomputed from a zero initial state after W warmup steps
matches the true trajectory to alpha^W (~2e-4 for alpha~0.12, W=4 — far
below the ~1e-2 bf16 matmul noise; chunk 0 is exact via an
atanh(h0)-alpha*h0 pad so h0 enters exactly).  T is split into independent
chunks of L=16 steps, each preceded by W warmup steps; all chunks advance
in lockstep, one vector MAC (scalar_tensor_tensor) + one scalar tanh per
step over a contiguous [128, G/2, C] tile (phase-major slot layout:
slot = s*C + c), h overwriting xw in place.

Layouts: channels e live on partitions (e = g*128 + p), time on the free
axis.  Host pre-transposes x and the weights so every matmul contracts on
partitions with zero on-device transposes:
    mm1: lhsT = W_in.T tile [d, e]   rhs = x.T [d, t]     -> psum [e, t]
    mm2: lhsT = W_x.T tile [e', e2]  rhs = x_proj [e', t] -> psum [e2, t]
    mm3: lhsT = cell tile [e, t]     rhs = W_out.T [e, n] -> psum [t, n]
Matmul I/O is bf16 (full PE rate); psum accumulation and the recurrence
state are fp32.
"""

import numpy as np
import ml_dtypes

# ---------------------------------------------------------------- constants
B = 8          # batch == number of cores
T = 2048       # sequence length
D = 1024       # channels
P = 128        # partitions
G = D // P     # channel groups (free-axis dim)
TP = 1024      # piece length (recurrence processed per piece)
L = 16         # chunk length
W = 4          # warmup steps
TB = 512       # matmul time-block
NP = T // TP   # pieces
C = TP // L    # chunks per piece (per group)
S = L + W      # lockstep steps per piece
PS = S * C     # xwh slot-columns per piece (phase-major: slot = s*C + c)
BUFC = NP * PS

_nc_cache = {}


def _set_dims(T_=2048, TP_=1024, L_=16, W_=4, TB_=512):
    """Recompute module dims — used by test.py for scaled-down validation.
    kernel() always runs the full problem size."""
    global T, TP, L, W, TB, NP, C, S, PS, BUFC
    T, TP, L, W, TB = T_, TP_, L_, W_, TB_
    NP = T // TP
    C = TP // L
    S = L + W
    PS = S * C
    BUFC = NP * PS
    _nc_cache.clear()


def _split_drain_waits(nc, mybir, max_waits=1):
    """This walrus build rejects instructions carrying more than a couple of
    sync waits ("Too many sync wait commands").  Move excess waits onto
    single-wait same-engine NOPs preceding the instruction — engines execute
    their queue in order, so waiting on a preceding NOP is equivalent."""
    for f in nc.m.functions:
        for bb in f.blocks:
            newlist = []
            for ins in bb.instructions:
                si = getattr(ins, "sync_info", None)
                if si is not None and len(si.on_wait) > max_waits:
                    waits = list(si.on_wait)
                    keep = waits[-max_waits:] if ins.opcode != "Drain" else []
                    spill = waits[:-max_waits] if ins.opcode != "Drain" else waits
                    si.on_wait.clear()
                    si.on_wait.extend(keep)
                    for i, w in enumerate(spill):
                        newlist.append(mybir.InstNoOp(
                            name=f"{ins.name}_waitsplit_{i}",
                            engine=ins.engine,
                            ins=[], outs=[],
                            sync_info=mybir.SyncInfo(on_wait=[w], on_update=[]),
                        ))
                newlist.append(ins)
            bb.instructions[:] = newlist


def build_nc(alpha_uniform=True, reps=1):
    """Build the per-core Bass program (same NEFF on all 8 cores).
    reps>1 wraps the whole pipeline in a hardware loop — used only for
    timing (HW time = (wall(reps=N) - wall(reps=1)) / (N-1))."""
    import concourse.bass as bass
    import concourse.mybir as mybir
    import concourse.tile as tile
    import contextlib

    key = ("v1", alpha_uniform, reps)
    if key in _nc_cache:
        return _nc_cache[key]

    f32 = mybir.dt.float32
    bf16 = mybir.dt.bfloat16
    AF = mybir.ActivationFunctionType
    OP = mybir.AluOpType

    nc = bass.Bass()

    # x arrives as [piece, t-block, p, g, t-within-block] (host-baked
    # layout) so each t-block's x loads with one linear DMA
    x_t = nc.dram_tensor("x_t", [NP, TP // TB, P, G, TB], bf16,
                         kind="ExternalInput")
    # weights arrive pre-arranged in SBUF layout [p, g, e] (host bakes the
    # (g p) e -> p g e rearrange) so the load is one linear DMA
    w_in = nc.dram_tensor("w_in", [P, G, D], bf16, kind="ExternalInput")    # W_in.T
    w_x = nc.dram_tensor("w_x", [P, G, D], bf16, kind="ExternalInput")      # W_x.T
    w_out = nc.dram_tensor("w_out", [P, G, D], bf16, kind="ExternalInput")  # W_out.T
    alpha_in = nc.dram_tensor("alpha_in", [P, 1] if alpha_uniform else [P, G],
                              f32, kind="ExternalInput")
    bias_b = nc.dram_tensor("bias_b", [P, G], f32, kind="ExternalInput")
    bias_g = nc.dram_tensor("bias_g", [P, G], f32, kind="ExternalInput")
    init0 = nc.dram_tensor("init0", [P, G, C], f32, kind="ExternalInput")
    pad0 = nc.dram_tensor("pad0", [P, G, W], f32, kind="ExternalInput")
    out = nc.dram_tensor("out", [T, D], f32, kind="ExternalOutput")
    h_fin = nc.dram_tensor("h_fin", [D], f32, kind="ExternalOutput")

    w_in3, w_x3, w_out3 = w_in, w_x, w_out

    with tile.TileContext(nc) as tc:
        with (
            tc.tile_pool(name="big", bufs=1) as big,
            tc.tile_pool(name="wpool", bufs=1) as wpool,
            tc.tile_pool(name="xin", bufs=3) as xin,
            tc.tile_pool(name="small", bufs=1) as small,
            tc.tile_pool(name="tmps", bufs=4) as tmps,
            tc.tile_pool(name="ostg", bufs=3) as ostg,
            tc.tile_pool(name="psum", bufs=8, space="PSUM") as psum,
        ):
            # persistent SBUF state
            xwh = big.tile([P, G, BUFC], f32)       # xw+b, overwritten by h in place
            gbuf = big.tile([P, G, T], bf16)        # x_proj -> silu gate -> cell
            w_in_sb = wpool.tile([P, G, D], bf16)
            w_x_sb = wpool.tile([P, G, D], bf16)
            w_out_sb = wpool.tile([P, G, D], bf16)
            alpha_sb = small.tile([P, 1] if alpha_uniform else [P, G], f32)
            bb_sb = small.tile([P, G], f32)
            bg_sb = small.tile([P, G], f32)
            init0_sb = small.tile([P, G, C], f32)
            zinit_sb = small.tile([P, G, C], f32)

            nc.sync.dma_start(w_in_sb[:], w_in3[:])
            nc.sync.dma_start(alpha_sb[:], alpha_in[:])
            nc.sync.dma_start(bb_sb[:], bias_b[:])
            nc.sync.dma_start(bg_sb[:], bias_g[:])
            nc.sync.dma_start(init0_sb[:], init0[:])
            # pad columns for piece-0 chunk-0 warmup: slots (s, c=0), s<W
            pad_sb = small.tile([P, G, W], f32)
            nc.sync.dma_start(pad_sb[:], pad0[:])
            nc.vector.tensor_copy(xwh[:, :, 0:W * C:C], pad_sb[:])
            nc.vector.memset(zinit_sb[:], 0.0)
            nc.sync.dma_start(w_x_sb[:], w_x3[:])
            nc.sync.dma_start(w_out_sb[:], w_out3[:])

            # Warm the PE HAM clock gate during the startup DMA wait:
            # ~4.5us of dummy matmuls on a zeroed stub lift the PE to
            # 2.4GHz before the first real matmul issues (cold MMs run at
            # 1.2GHz for the first ~3.4us otherwise).  Results land in a
            # rotating psum slot and are never read.
            wstub = small.tile([P, P], bf16)
            nc.vector.memset(wstub[:], 0.0)
            for _ in range(16):
                wps = psum.tile([P, 512], f32, tag="ps")
                for _ in range(8):
                    nc.tensor.matmul(wps[:, 0:P], wstub[:], wstub[:],
                                     start=True, stop=True)

            def recur_units(p):
                # All C chunks advance in lockstep over slot columns
                # (slot = s*C + c, phase-major, so every access below is
                # contiguous).  Each slot is written exactly once: xw/pad by
                # the evictions/warmup copies, then h in place by the tanh.
                pbase = p * PS
                init_sb = init0_sb if p == 0 else zinit_sb
                for s in range(S):
                    for chain in range(2):
                        gs = slice(chain * (G // 2), (chain + 1) * (G // 2))
                        cur = xwh[:, gs, pbase + s * C: pbase + (s + 1) * C]
                        if s == 0:
                            prev = init_sb[:, gs, :]
                        else:
                            prev = xwh[:, gs, pbase + (s - 1) * C: pbase + s * C]
                        if alpha_uniform:
                            nc.vector.scalar_tensor_tensor(
                                cur, prev, alpha_sb[:, 0:1], cur,
                                OP.mult, OP.add)
                        else:
                            tmp = tmps.tile([P, G // 2, C], f32, tag=f"tmp{chain}")
                            ab = alpha_sb[:, gs, None].to_broadcast((P, G // 2, C))
                            nc.vector.tensor_tensor(tmp[:], prev, ab, OP.mult)
                            nc.vector.tensor_tensor(cur, tmp[:], cur, OP.add)
                        nc.scalar.activation(cur, cur, AF.Tanh)
                    yield

            def gslot(p, dgs=None):
                # gbuf viewed as slots: [P, (g), b, c] with b = t%L, c = chunk
                if dgs is None:
                    v = gbuf[:, :, p * TP:(p + 1) * TP]
                    return v.rearrange("p g (b c) -> p g b c", c=C)
                v = gbuf[:, dgs, p * TP:(p + 1) * TP]
                return v.rearrange("p (b c) -> p b c", c=C)

            def mm12_units(p):
                # mm1 (x_proj -> gbuf slots) and mm2 (xw + b -> xwh slots)
                pbase = p * PS
                AB = TB // L          # chunks per t-block
                for tb in range(TP // TB):
                    xs = xin.tile([P, G, TB], bf16, tag="xs")
                    # gpsimd SWDGE path: keeps x loads off the sync queue,
                    # which is busy with weight DMAs at startup
                    nc.gpsimd.dma_start(xs[:], x_t[p, tb])
                    for eg in range(G):
                        ps = psum.tile([P, TB], f32, tag="ps")
                        for dg in range(G):
                            nc.tensor.matmul(
                                ps[:], w_in_sb[:, dg, eg * P:(eg + 1) * P],
                                xs[:, dg, :],
                                start=(dg == 0), stop=(dg == G - 1))
                        # scramble t-order psum into slot-order gbuf
                        nc.vector.tensor_copy(
                            gslot(p, eg)[:, :, tb * AB:(tb + 1) * AB],
                            ps[:].rearrange("p (a b) -> p b a", b=L))
                        yield
                    for eg in range(G):
                        ps = psum.tile([P, TB], f32, tag="ps")
                        for dg in range(G):
                            nc.tensor.matmul(
                                ps[:], w_x_sb[:, dg, eg * P:(eg + 1) * P],
                                gslot(p, dg)[:, :, tb * AB:(tb + 1) * AB],
                                start=(dg == 0), stop=(dg == G - 1))
                        # psum columns already slot-ordered; store to data slots
                        dst = xwh[:, eg, pbase + W * C: pbase + S * C]
                        dst = dst.rearrange("p (b c) -> p b c", c=C)
                        nc.vector.tensor_scalar(
                            dst[:, :, tb * AB:(tb + 1) * AB],
                            ps[:].rearrange("p (b a) -> p b a", a=AB),
                            bb_sb[:, eg:eg + 1], None, OP.add)
                        yield
                # fill warmup slots (s<W) of this piece from its own xw data,
                # and chunk-0 warmup of the NEXT piece from this piece's tail
                # (both must happen before the respective recurrences
                # overwrite the source slots with h)
                vw = xwh[:, :, pbase: pbase + PS]
                vw = vw.rearrange("p g (s c) -> p g s c", c=C)
                nc.vector.tensor_copy(vw[:, :, 0:W, 1:C],
                                      vw[:, :, L:L + W, 0:C - 1])
                if p + 1 < NP:
                    nx = xwh[:, :, (p + 1) * PS: (p + 2) * PS]
                    nx = nx.rearrange("p g (s c) -> p g s c", c=C)
                    nc.vector.tensor_copy(nx[:, :, 0:W, 0:1],
                                          vw[:, :, L:L + W, C - 1:C])

            def silu_units(p):
                # gate part 1: gbuf <- silu(x_proj + b_gate); per-group so the
                # per-partition bias slice is uniform
                for gg in range(G):
                    nc.scalar.activation(
                        gbuf[:, gg, p * TP:(p + 1) * TP],
                        gbuf[:, gg, p * TP:(p + 1) * TP],
                        AF.Silu, bias=bg_sb[:, gg:gg + 1])
                    yield

            def mm3_units(p):
                # cell = h * gate (contiguous in slot space, per block so PE
                # starts right after the first block), then out = cell@W_out.T
                # with slot-contiguous lhsT (stationary APs must have a single
                # free dim).  Output rows land in DRAM in slot order; the host
                # unscrambles with a cheap gather.  Evictions alternate V/S.
                pbase = p * PS
                GB = min(2 * P, TP)   # gate block (slots)
                for mt in range(TP // P):
                    k0 = mt * P
                    if k0 % GB == 0:
                        nc.vector.tensor_tensor(
                            gbuf[:, :, p * TP + k0: p * TP + k0 + GB],
                            xwh[:, :, pbase + W * C + k0: pbase + W * C + k0 + GB],
                            gbuf[:, :, p * TP + k0: p * TP + k0 + GB], OP.mult)
                    for nh in range(D // 512):
                        ps = psum.tile([P, 512], f32, tag="ps")
                        for eg in range(G):
                            nc.tensor.matmul(
                                ps[:], gbuf[:, eg, p * TP + k0: p * TP + k0 + P],
                                w_out_sb[:, eg, nh * 512:(nh + 1) * 512],
                                start=(eg == 0), stop=(eg == G - 1))
                        st = ostg.tile([P, 512], f32, tag="ost")
                        if (mt + nh) % 2 == 0:
                            nc.scalar.copy(st[:], ps[:])
                        else:
                            nc.vector.tensor_copy(st[:], ps[:])
                        r0 = p * TP + mt * P
                        nc.sync.dma_start(
                            out[r0:r0 + P, nh * 512:(nh + 1) * 512], st[:])
                        yield

            def pump(gen, n):
                for _ in range(n):
                    if next(gen, "done") == "done":
                        return True
                return False

            loop_cm = tc.For_i(0, reps, 1) if reps > 1 else contextlib.nullcontext()
            with loop_cm:
                # Software pipeline across pieces: PE stream is
                #   mm12(0), mm12(1), mm3(0), mm12(2), mm3(1), ..., mm3(NP-1)
                # while piece p's recurrence/silu (V+S) runs during the PE
                # work emitted alongside it — PE never waits on a recurrence.
                for _ in mm12_units(0):
                    pass
                for p in range(NP):
                    rec = recur_units(p)
                    filler = []
                    if p >= 1:
                        filler.append(mm3_units(p - 1))
                    if p + 1 < NP:
                        filler.append(mm12_units(p + 1))
                    sil = silu_units(p)
                    # interleave: pump a few recurrence steps per filler unit
                    total_filler = (TP // P) * (D // 512) * (1 if p >= 1 else 0) \
                        + 2 * G * (TP // TB) * (1 if p + 1 < NP else 0)
                    per = max(1, -(-S // max(total_filler, 1)))
                    done = False
                    for fg in filler:
                        for _ in fg:
                            if not done:
                                done = pump(rec, per)
                    while not done:
                        done = pump(rec, S)
                    # silu can start as soon as mm2(p) has consumed gbuf
                    for _ in sil:
                        pass
                for _ in mm3_units(NP - 1):
                    pass

            # final hidden state: last valid h column
            nc.sync.dma_start(
                h_fin.rearrange("(g p) -> p g", p=P)[:, :, None],
                xwh[:, :, BUFC - 1:BUFC])

    _split_drain_waits(nc, mybir, max_waits=1)
    _nc_cache[key] = nc
    return nc


def _prep_inputs(x, h0, W_in, W_x, alpha_raw, b, b_gate, W_out):
    """Host-side sharding/layout prep. Returns (in_maps, perm, alpha_uniform)."""
    x = np.asarray(x, np.float32)
    h0 = np.asarray(h0, np.float32)
    W_in = np.asarray(W_in, np.float32)
    W_x = np.asarray(W_x, np.float32)
    alpha_raw = np.asarray(alpha_raw, np.float32)
    b = np.asarray(b, np.float32)
    b_gate = np.asarray(b_gate, np.float32)
    W_out = np.asarray(W_out, np.float32)

    alpha = 1.0 / (1.0 + np.exp(-alpha_raw.astype(np.float64)))
    alpha = alpha.astype(np.float32)

    # Channel permutation so alpha is constant per partition (e = g*128 + p):
    # needed for the single-instruction MAC with a per-partition scalar.
    # alpha[g*128 + p] must be independent of g for each p.
    def uniform_ok(a):
        r = a.reshape(G, P)
        return bool(np.all(r == r[0:1]))

    if uniform_ok(alpha):
        perm = np.arange(D)
        alpha_uniform = True
    else:
        perm = np.argsort(alpha, kind="stable")
        # sorted values laid out p-major: position k -> (p = k // G, g = k % G)
        a_s = alpha[perm]
        if np.all(a_s.reshape(P, G) == a_s.reshape(P, G)[:, 0:1]):
            # reorder perm so that channel e' = g*128 + p gets sorted[p*G + g]
            perm = perm.reshape(P, G).T.reshape(D)
            alpha_uniform = True
        else:
            perm = np.arange(D)
            alpha_uniform = False

    alpha_p = alpha[perm]
    W_in_p = W_in[perm, :]
    W_x_p = W_x[perm, :][:, perm]
    W_out_p = W_out[:, perm]
    b_p = b[perm]
    bg_p = b_gate[perm]
    h0_p = h0[:, perm]

    bf = ml_dtypes.bfloat16

    def wlayout(wT):
        # [d, e] -> [p, g, e] with d = g*P + p, contiguous
        return np.ascontiguousarray(
            wT.reshape(G, P, D).swapaxes(0, 1)).astype(bf)

    w_in_in = wlayout(W_in_p.T)
    w_x_in = wlayout(W_x_p.T)
    w_out_in = wlayout(W_out_p.T)

    if alpha_uniform:
        alpha_dev = np.ascontiguousarray(alpha_p.reshape(G, P).T[:, 0:1])
    else:
        alpha_dev = np.ascontiguousarray(alpha_p.reshape(G, P).T)
    bb_dev = np.ascontiguousarray(b_p.reshape(G, P).T)
    bg_dev = np.ascontiguousarray(bg_p.reshape(G, P).T)

    in_maps = []
    for k in range(B):
        xt = np.ascontiguousarray(x[k].T).astype(bf)          # [D, T]
        nTB = TP // TB
        xt = xt.reshape(G, P, NP, nTB, TB).transpose(2, 3, 1, 0, 4)
        x_t = np.ascontiguousarray(xt)                        # [NP,nTB,P,G,TB]
        h0k = h0_p[k]
        init0 = np.zeros((P, G, C), np.float32)
        init0[:, :, 0] = h0k.reshape(G, P).T
        h0c = np.clip(h0k, -0.9999999, 0.9999999)
        pad_val = np.arctanh(h0c) - alpha_p * h0k          # [D]
        pad0 = np.repeat(pad_val.reshape(G, P).T[:, :, None], W, axis=2)
        in_maps.append({
            "x_t": x_t,
            "w_in": w_in_in, "w_x": w_x_in, "w_out": w_out_in,
            "alpha_in": np.ascontiguousarray(alpha_dev, np.float32),
            "bias_b": np.ascontiguousarray(bb_dev, np.float32),
            "bias_g": np.ascontiguousarray(bg_dev, np.float32),
            "init0": init0,
            "pad0": np.ascontiguousarray(pad0, np.float32),
        })
    return in_maps, perm, alpha_uniform


def kernel(x, h0, W_in, W_x, alpha_raw, b, b_gate, W_out):
    from concourse.bass_utils import run_bass_kernel_spmd

    in_maps, perm, alpha_uniform = _prep_inputs(
        x, h0, W_in, W_x, alpha_raw, b, b_gate, W_out)
    nc = build_nc(alpha_uniform=alpha_uniform)
    res = run_bass_kernel_spmd(nc, in_maps, core_ids=list(range(B)))

    # device "out" rows are in slot order: row p*TP + b*C + c holds
    # t = p*TP + c*L + b.  Build the gather index t -> row once.
    tt = np.arange(T)
    pp, rr = tt // TP, tt % TP
    rows = pp * TP + (rr % L) * C + rr // L
    output = np.stack([res.results[k]["out"][rows] for k in range(B)], axis=0)
    hf = np.empty((B, D), np.float32)
    for k in range(B):
        hf[k, perm] = res.results[k]["h_fin"]
    return np.ascontiguousarray(output, dtype=np.float32), hf
